# revision 21
# baseline (speedup 1.0000x reference)
"""Multi-head graph-attention layer for Trainium2 (8-core SPMD).

The reference computes per-head projections hp = einsum("bnf,hfd->bhnd", h, W),
dense attention scores e = hp @ hp^T, LeakyReLU, softmax over the last axis,
and then multiplies hp by sum_j(softmax(e))_j. The sum of a softmax over its
own normalization axis is identically 1, so the layer's exact mathematical
output is hp itself (concatenated over heads):

    out[b, n, h*64+d] = sum_f h[b,n,f] * W[h,f,d]  =  (h[b] @ Wc)[n, h*64+d]

with Wc[f, h*64+d] = W[h,f,d]. The reference's deviation from rowsum==1 is
fp32 rounding noise (~1e-6 relative) that no reimplementation reproduces, so
computing the projection directly is both the fastest and the most accurate
realization. `adj` is unused by the reference and is ignored here.

Sharding: data-parallel over the batch dim B=8, one graph per NeuronCore.
Each core computes Y[b]^T = (Wc^T @ h[b]^T) as a [256,256] x [256,2048]
matmul with Wc chunks stationary on the PE (float32r: single-pass reduced-
precision fp32, 1 cycle/row). Inputs are host-transposed to [F_IN, N] so
every DMA is fully contiguous. Warm-up matmuls run on scratch data during
the input-DMA wait so the real matmuls hit the 2.4 GHz warm clock.
"""

import numpy as np

import concourse.bass as bass
import concourse.mybir as mybir
import concourse.tile as tile
from concourse import bacc
from concourse.bass_utils import run_bass_kernel_spmd

B = 8          # graphs == cores
N = 2048       # nodes per graph
F_IN = 256     # input features (= contraction dim K)
F_OUT = 256    # num_heads * d_head
P = 128        # SBUF/PSUM partitions
NTILE = 512    # PSUM bank free-dim (fp32)

KC = F_IN // P     # 2 contraction chunks
MC = F_OUT // P    # 2 output-feature chunks
NC_ = N // NTILE   # 4 node chunks
XSPLIT = 2         # node-dim halves per x DMA
XW = N // XSPLIT   # 1024

N_WARMUP_MM = 11   # scratch matmuls covering the input-DMA wait

# PE matmul dtype: float32 (exact, 4 cycles/row), float32r (reduced-precision
# single pass, 1 cycle/row at N=512, rel err ~1.4e-4), bfloat16 (1 cycle/row,
# half input DMA, rel err ~2.2e-3).
MATMUL_DTYPE = "float32r"

_module_cache = {}

# test.py reads this after calling kernel() to get profile/exec-time info.
LAST_RESULTS = None


def _build_module(mm_dtype: str) -> bass.Bass:
    if mm_dtype == "bfloat16":
        in_dt = mybir.dt.bfloat16
    elif mm_dtype == "float32r":
        in_dt = mybir.dt.float32r
    else:
        in_dt = mybir.dt.float32

    nc = bacc.Bacc(None, target_bir_lowering=False)
    # Host-packed input: xin[f, 0:256] = Wc[f, :], xin[f, 256:] = X^T[f, :].
    xin = nc.dram_tensor("xin", [F_IN, F_OUT + N], in_dt, kind="ExternalInput")
    yt = nc.dram_tensor("yt", [F_OUT, N], mybir.dt.float32, kind="ExternalOutput")
    XOFF = F_OUT  # x columns start here inside a packed row

    with tile.TileContext(nc) as tc:
        with (
            tc.tile_pool(name="wpool", bufs=1) as wpool,
            tc.tile_pool(name="xpool", bufs=1) as xpool,
            tc.tile_pool(name="ypool", bufs=1) as ypool,
            tc.tile_pool(name="warmpool", bufs=1) as warmpool,
            tc.tile_pool(name="drampool", bufs=1, space="DRAM") as dram,
            tc.tile_pool(name="pspool", bufs=1, space="PSUM") as pspool,
        ):
            # Scratch operands for PE warm-up (zeros; values are irrelevant).
            wu = warmpool.tile([P, NTILE], mybir.dt.bfloat16, name="wu", tag="wu")
            nc.gpsimd.memset(wu[:], 0.0)
            wu_mm = wu[:]

            # Per-k packed tiles [128, 2304]: cols 0:256 weights, 256: x.
            # Two single-run DMAs per k so matmuls chase the stream; all on
            # the sync HWDGE queue in need-order.
            CUT = F_OUT + XW  # first DMA carries w + first x half
            xk_sb = [
                xpool.tile([P, F_OUT + N], in_dt, name=f"xk{k}", tag=f"xk{k}")
                for k in range(KC)
            ]
            for k in range(KC):
                nc.sync.dma_start(
                    xk_sb[k][:, :CUT], xin[k * P : (k + 1) * P, :CUT]
                )
                nc.sync.dma_start(
                    xk_sb[k][:, CUT:], xin[k * P : (k + 1) * P, CUT:]
                )


            ps = [
                [
                    pspool.tile(
                        [P, NTILE], mybir.dt.float32, name=f"ps{m}_{n}", tag=f"ps{m}_{n}"
                    )
                    for n in range(NC_)
                ]
                for m in range(MC)
            ]
            y_sb = [
                ypool.tile([P, N], mybir.dt.float32, name=f"y{m}", tag=f"y{m}")
                for m in range(MC)
            ]

            # PE clock warm-up on scratch data while the x DMAs are in
            # flight. Runs on ps[0][0] before its real accumulation group;
            # Tile's WAW tracking keeps program order.
            for _ in range(N_WARMUP_MM):
                nc.tensor.matmul(ps[0][0][:], wu_mm[:, :P], wu_mm, start=True, stop=True)

            # k-outer, node-chunk inner ordered to chase the input stream:
            # k=0 matmuls run on each x0 half as it lands, k=1 passes close
            # each accumulation group, whose chunk then evicts + flies out.
            for k in range(KC):
                for n in range(NC_):
                    for m in range(MC):
                        nc.tensor.matmul(
                            ps[m][n][:],
                            xk_sb[k][:, m * P : (m + 1) * P],
                            xk_sb[k][:, XOFF + n * NTILE : XOFF + (n + 1) * NTILE],
                            start=(k == 0),
                            stop=(k == KC - 1),
                        )
                        if k == KC - 1:
                            # Eviction alternates DVE/ACT; all outputs go on
                            # the single warmed-up HWDGE stream.
                            dst = y_sb[m][:, n * NTILE : (n + 1) * NTILE]
                            if (2 * n + m) % 2 == 0:
                                nc.vector.tensor_copy(dst, ps[m][n][:])
                            else:
                                nc.scalar.copy(dst, ps[m][n][:])
                            nc.scalar.dma_start(
                                yt[m * P : (m + 1) * P, n * NTILE : (n + 1) * NTILE],
                                dst,
                            )
    nc.compile()
    return nc


def _get_module() -> bass.Bass:
    if MATMUL_DTYPE not in _module_cache:
        _module_cache[MATMUL_DTYPE] = _build_module(MATMUL_DTYPE)
    return _module_cache[MATMUL_DTYPE]


def kernel(h: np.ndarray, adj: np.ndarray, W: np.ndarray, **_unused) -> np.ndarray:
    global LAST_RESULTS
    h = np.asarray(h, dtype=np.float32)
    W = np.asarray(W, dtype=np.float32)
    # Wc[f, head*64+d] = W[head, f, d]
    wc = np.ascontiguousarray(W.transpose(1, 0, 2).reshape(F_IN, F_OUT))

    if MATMUL_DTYPE == "bfloat16":
        import ml_dtypes

        cast = lambda a: np.ascontiguousarray(a.astype(ml_dtypes.bfloat16))
    else:
        cast = np.ascontiguousarray

    wc_in = cast(wc)
    in_maps = [
        {"xin": np.ascontiguousarray(np.hstack([wc_in, cast(h[b].T)]))}
        for b in range(B)
    ]
    nc = _get_module()
    res = run_bass_kernel_spmd(nc, in_maps, core_ids=list(range(B)))
    LAST_RESULTS = res

    out = np.empty((B, N, F_OUT), dtype=np.float32)
    for b in range(B):
        out[b] = res.results[b]["yt"].T
    return out


# revision 22
# speedup vs baseline: 1.0865x; 1.0865x over previous
"""Multi-head graph-attention layer for Trainium2 (8-core SPMD).

The reference computes per-head projections hp = einsum("bnf,hfd->bhnd", h, W),
dense attention scores e = hp @ hp^T, LeakyReLU, softmax over the last axis,
and then multiplies hp by sum_j(softmax(e))_j. The sum of a softmax over its
own normalization axis is identically 1, so the layer's exact mathematical
output is hp itself (concatenated over heads):

    out[b, n, h*64+d] = sum_f h[b,n,f] * W[h,f,d]  =  (h[b] @ Wc)[n, h*64+d]

with Wc[f, h*64+d] = W[h,f,d]. The reference's deviation from rowsum==1 is
fp32 rounding noise (~1e-6 relative) that no reimplementation reproduces, so
computing the projection directly is both the fastest and the most accurate
realization. `adj` is unused by the reference and is ignored here.

Sharding: data-parallel over the batch dim B=8, one graph per NeuronCore.
Each core computes Y[b]^T = (Wc^T @ h[b]^T) as a [256,256] x [256,2048]
matmul with Wc chunks stationary on the PE (float32r: single-pass reduced-
precision fp32, 1 cycle/row). Inputs are host-transposed to [F_IN, N] so
every DMA is fully contiguous. Warm-up matmuls run on scratch data during
the input-DMA wait so the real matmuls hit the 2.4 GHz warm clock.
"""

import numpy as np

import concourse.bass as bass
import concourse.mybir as mybir
import concourse.tile as tile
from concourse import bacc
from concourse.bass_utils import run_bass_kernel_spmd

B = 8          # graphs == cores
N = 2048       # nodes per graph
F_IN = 256     # input features (= contraction dim K)
F_OUT = 256    # num_heads * d_head
P = 128        # SBUF/PSUM partitions
NTILE = 512    # PSUM bank free-dim (fp32)

KC = F_IN // P     # 2 contraction chunks
MC = F_OUT // P    # 2 output-feature chunks
NC_ = N // NTILE   # 4 node chunks
XSPLIT = 2         # node-dim halves per x DMA
XW = N // XSPLIT   # 1024

N_WARMUP_MM = 11   # scratch matmuls covering the input-DMA wait

# PE matmul dtype: float32 (exact, 4 cycles/row), float32r (reduced-precision
# single pass, 1 cycle/row at N=512, rel err ~1.4e-4), bfloat16 (1 cycle/row,
# half input DMA, rel err ~2.2e-3).
MATMUL_DTYPE = "float32r"

_module_cache = {}

# test.py reads this after calling kernel() to get profile/exec-time info.
LAST_RESULTS = None


def _build_module(mm_dtype: str) -> bass.Bass:
    if mm_dtype == "bfloat16":
        in_dt = mybir.dt.bfloat16
    elif mm_dtype == "float32r":
        in_dt = mybir.dt.float32r
    else:
        in_dt = mybir.dt.float32

    nc = bacc.Bacc(None, target_bir_lowering=False)
    # Host-packed input: xin[f, 0:256] = Wc[f, :], xin[f, 256:] = X^T[f, :].
    xin = nc.dram_tensor("xin", [F_IN, F_OUT + N], in_dt, kind="ExternalInput")
    yt = nc.dram_tensor("yt", [F_OUT, N], mybir.dt.float32, kind="ExternalOutput")
    XOFF = F_OUT  # x columns start here inside a packed row

    with tile.TileContext(nc) as tc:
        with (
            tc.tile_pool(name="wpool", bufs=1) as wpool,
            tc.tile_pool(name="xpool", bufs=1) as xpool,
            tc.tile_pool(name="ypool", bufs=1) as ypool,
            tc.tile_pool(name="warmpool", bufs=1) as warmpool,
            tc.tile_pool(name="drampool", bufs=1, space="DRAM") as dram,
            tc.tile_pool(name="pspool", bufs=1, space="PSUM") as pspool,
        ):
            # Scratch operands for PE warm-up (zeros; values are irrelevant).
            wu = warmpool.tile([P, NTILE], mybir.dt.bfloat16, name="wu", tag="wu")
            nc.gpsimd.memset(wu[:], 0.0)
            wu_mm = wu[:]

            # Per-k packed tiles [128, 2304]: cols 0:256 weights, 256: x.
            # Two single-run DMAs per k so matmuls chase the stream; all on
            # the sync HWDGE queue in need-order.
            CUT = F_OUT + XW  # k=1 is split: w + first x half, then rest
            xk_sb = [
                xpool.tile([P, F_OUT + N], in_dt, name=f"xk{k}", tag=f"xk{k}")
                for k in range(KC)
            ]
            nc.sync.dma_start(xk_sb[0][:], xin[0:P, :])
            nc.sync.dma_start(xk_sb[1][:, :CUT], xin[P : 2 * P, :CUT])
            nc.sync.dma_start(xk_sb[1][:, CUT:], xin[P : 2 * P, CUT:])


            ps = [
                [
                    pspool.tile(
                        [P, NTILE], mybir.dt.float32, name=f"ps{m}_{n}", tag=f"ps{m}_{n}"
                    )
                    for n in range(NC_)
                ]
                for m in range(MC)
            ]
            y_sb = [
                ypool.tile([P, N], mybir.dt.float32, name=f"y{m}", tag=f"y{m}")
                for m in range(MC)
            ]

            # PE clock warm-up on scratch data while the x DMAs are in
            # flight. Runs on ps[0][0] before its real accumulation group;
            # Tile's WAW tracking keeps program order.
            for _ in range(N_WARMUP_MM):
                nc.tensor.matmul(ps[0][0][:], wu_mm[:, :P], wu_mm, start=True, stop=True)

            # k-outer, node-chunk inner ordered to chase the input stream:
            # k=0 matmuls run on each x0 half as it lands, k=1 passes close
            # each accumulation group, whose chunk then evicts + flies out.
            for k in range(KC):
                for n in range(NC_):
                    for m in range(MC):
                        nc.tensor.matmul(
                            ps[m][n][:],
                            xk_sb[k][:, m * P : (m + 1) * P],
                            xk_sb[k][:, XOFF + n * NTILE : XOFF + (n + 1) * NTILE],
                            start=(k == 0),
                            stop=(k == KC - 1),
                        )
                        if k == KC - 1:
                            # Eviction alternates DVE/ACT; all outputs go on
                            # the single warmed-up HWDGE stream.
                            dst = y_sb[m][:, n * NTILE : (n + 1) * NTILE]
                            if (2 * n + m) % 2 == 0:
                                nc.vector.tensor_copy(dst, ps[m][n][:])
                            else:
                                nc.scalar.copy(dst, ps[m][n][:])
                            nc.sync.dma_start(
                                yt[m * P : (m + 1) * P, n * NTILE : (n + 1) * NTILE],
                                dst,
                            )
    nc.compile()
    return nc


def _get_module() -> bass.Bass:
    if MATMUL_DTYPE not in _module_cache:
        _module_cache[MATMUL_DTYPE] = _build_module(MATMUL_DTYPE)
    return _module_cache[MATMUL_DTYPE]


def kernel(h: np.ndarray, adj: np.ndarray, W: np.ndarray, **_unused) -> np.ndarray:
    global LAST_RESULTS
    h = np.asarray(h, dtype=np.float32)
    W = np.asarray(W, dtype=np.float32)
    # Wc[f, head*64+d] = W[head, f, d]
    wc = np.ascontiguousarray(W.transpose(1, 0, 2).reshape(F_IN, F_OUT))

    if MATMUL_DTYPE == "bfloat16":
        import ml_dtypes

        cast = lambda a: np.ascontiguousarray(a.astype(ml_dtypes.bfloat16))
    else:
        cast = np.ascontiguousarray

    wc_in = cast(wc)
    in_maps = [
        {"xin": np.ascontiguousarray(np.hstack([wc_in, cast(h[b].T)]))}
        for b in range(B)
    ]
    nc = _get_module()
    res = run_bass_kernel_spmd(nc, in_maps, core_ids=list(range(B)))
    LAST_RESULTS = res

    out = np.empty((B, N, F_OUT), dtype=np.float32)
    for b in range(B):
        out[b] = res.results[b]["yt"].T
    return out


# revision 23
# speedup vs baseline: 1.1068x; 1.0187x over previous
"""Multi-head graph-attention layer for Trainium2 (8-core SPMD).

The reference computes per-head projections hp = einsum("bnf,hfd->bhnd", h, W),
dense attention scores e = hp @ hp^T, LeakyReLU, softmax over the last axis,
and then multiplies hp by sum_j(softmax(e))_j. The sum of a softmax over its
own normalization axis is identically 1, so the layer's exact mathematical
output is hp itself (concatenated over heads):

    out[b, n, h*64+d] = sum_f h[b,n,f] * W[h,f,d]  =  (h[b] @ Wc)[n, h*64+d]

with Wc[f, h*64+d] = W[h,f,d]. The reference's deviation from rowsum==1 is
fp32 rounding noise (~1e-6 relative) that no reimplementation reproduces, so
computing the projection directly is both the fastest and the most accurate
realization. `adj` is unused by the reference and is ignored here.

Sharding: data-parallel over the batch dim B=8, one graph per NeuronCore.
Each core computes Y[b]^T = (Wc^T @ h[b]^T) as a [256,256] x [256,2048]
matmul with Wc chunks stationary on the PE (float32r: single-pass reduced-
precision fp32, 1 cycle/row). Inputs are host-transposed to [F_IN, N] so
every DMA is fully contiguous. Warm-up matmuls run on scratch data during
the input-DMA wait so the real matmuls hit the 2.4 GHz warm clock.
"""

import numpy as np

import concourse.bass as bass
import concourse.mybir as mybir
import concourse.tile as tile
from concourse import bacc
from concourse.bass_utils import run_bass_kernel_spmd

B = 8          # graphs == cores
N = 2048       # nodes per graph
F_IN = 256     # input features (= contraction dim K)
F_OUT = 256    # num_heads * d_head
P = 128        # SBUF/PSUM partitions
NTILE = 512    # PSUM bank free-dim (fp32)

KC = F_IN // P     # 2 contraction chunks
MC = F_OUT // P    # 2 output-feature chunks
NC_ = N // NTILE   # 4 node chunks
XSPLIT = 2         # node-dim halves per x DMA
XW = N // XSPLIT   # 1024

N_WARMUP_MM = 11   # scratch matmuls covering the input-DMA wait

# PE matmul dtype: float32 (exact, 4 cycles/row), float32r (reduced-precision
# single pass, 1 cycle/row at N=512, rel err ~1.4e-4), bfloat16 (1 cycle/row,
# half input DMA, rel err ~2.2e-3).
MATMUL_DTYPE = "float32r"

_module_cache = {}

# test.py reads this after calling kernel() to get profile/exec-time info.
LAST_RESULTS = None


def _build_module(mm_dtype: str) -> bass.Bass:
    if mm_dtype == "bfloat16":
        in_dt = mybir.dt.bfloat16
    elif mm_dtype == "float32r":
        in_dt = mybir.dt.float32r
    else:
        in_dt = mybir.dt.float32

    nc = bacc.Bacc(None, target_bir_lowering=False)
    # Host-packed input: xin[f, 0:256] = Wc[f, :], xin[f, 256:] = X^T[f, :].
    xin = nc.dram_tensor("xin", [F_IN, F_OUT + N], in_dt, kind="ExternalInput")
    yt = nc.dram_tensor("yt", [F_OUT, N], mybir.dt.float32, kind="ExternalOutput")
    XOFF = F_OUT  # x columns start here inside a packed row

    with tile.TileContext(nc) as tc:
        with (
            tc.tile_pool(name="wpool", bufs=1) as wpool,
            tc.tile_pool(name="xpool", bufs=1) as xpool,
            tc.tile_pool(name="ypool", bufs=1) as ypool,
            tc.tile_pool(name="warmpool", bufs=1) as warmpool,
            tc.tile_pool(name="drampool", bufs=1, space="DRAM") as dram,
            tc.tile_pool(name="pspool", bufs=1, space="PSUM") as pspool,
        ):
            # Scratch operands for PE warm-up (zeros; values are irrelevant).
            wu = warmpool.tile([P, NTILE], mybir.dt.bfloat16, name="wu", tag="wu")
            nc.gpsimd.memset(wu[:], 0.0)
            wu_mm = wu[:]

            # Per-k packed tiles [128, 2304]: cols 0:256 weights, 256: x.
            # Two single-run DMAs per k so matmuls chase the stream; all on
            # the sync HWDGE queue in need-order.
            CUT = F_OUT + XW  # k=1 is split: w + first x half, then rest
            xk_sb = [
                xpool.tile([P, F_OUT + N], in_dt, name=f"xk{k}", tag=f"xk{k}")
                for k in range(KC)
            ]
            nc.sync.dma_start(xk_sb[0][:], xin[0:P, :])
            nc.sync.dma_start(xk_sb[1][:, :CUT], xin[P : 2 * P, :CUT])
            nc.sync.dma_start(xk_sb[1][:, CUT:], xin[P : 2 * P, CUT:])


            ps = [
                [
                    pspool.tile(
                        [P, NTILE], mybir.dt.float32, name=f"ps{m}_{n}", tag=f"ps{m}_{n}"
                    )
                    for n in range(NC_)
                ]
                for m in range(MC)
            ]
            y_sb = [
                ypool.tile([P, N], mybir.dt.float32, name=f"y{m}", tag=f"y{m}")
                for m in range(MC)
            ]

            # PE clock warm-up on scratch data while the x DMAs are in
            # flight. Runs on ps[0][0] before its real accumulation group;
            # Tile's WAW tracking keeps program order.
            for _ in range(N_WARMUP_MM):
                nc.tensor.matmul(ps[0][0][:], wu_mm[:, :P], wu_mm, start=True, stop=True)

            # k-outer, node-chunk inner ordered to chase the input stream:
            # k=0 matmuls run on each x0 half as it lands, k=1 passes close
            # each accumulation group, whose chunk then evicts + flies out.
            for k in range(KC):
                for n in range(NC_):
                    for m in range(MC):
                        nc.tensor.matmul(
                            ps[m][n][:],
                            xk_sb[k][:, m * P : (m + 1) * P],
                            xk_sb[k][:, XOFF + n * NTILE : XOFF + (n + 1) * NTILE],
                            start=(k == 0),
                            stop=(k == KC - 1),
                        )
                        if k == KC - 1:
                            # Eviction alternates DVE/ACT; all outputs go on
                            # the single warmed-up HWDGE stream.
                            dst = y_sb[m][:, n * NTILE : (n + 1) * NTILE]
                            yslice = yt[m * P : (m + 1) * P, n * NTILE : (n + 1) * NTILE]
                            if (2 * n + m) % 2 == 0:
                                # DVE evicts; the idle sync engine issues the
                                # store on its (ramped) queue.
                                nc.vector.tensor_copy(dst, ps[m][n][:])
                                nc.sync.dma_start(yslice, dst)
                            else:
                                # ACT evicts and issues its own store on the
                                # scalar queue - no cross-engine stall.
                                nc.scalar.copy(dst, ps[m][n][:])
                                nc.scalar.dma_start(yslice, dst)
    nc.compile()
    return nc


def _get_module() -> bass.Bass:
    if MATMUL_DTYPE not in _module_cache:
        _module_cache[MATMUL_DTYPE] = _build_module(MATMUL_DTYPE)
    return _module_cache[MATMUL_DTYPE]


def kernel(h: np.ndarray, adj: np.ndarray, W: np.ndarray, **_unused) -> np.ndarray:
    global LAST_RESULTS
    h = np.asarray(h, dtype=np.float32)
    W = np.asarray(W, dtype=np.float32)
    # Wc[f, head*64+d] = W[head, f, d]
    wc = np.ascontiguousarray(W.transpose(1, 0, 2).reshape(F_IN, F_OUT))

    if MATMUL_DTYPE == "bfloat16":
        import ml_dtypes

        cast = lambda a: np.ascontiguousarray(a.astype(ml_dtypes.bfloat16))
    else:
        cast = np.ascontiguousarray

    wc_in = cast(wc)
    in_maps = [
        {"xin": np.ascontiguousarray(np.hstack([wc_in, cast(h[b].T)]))}
        for b in range(B)
    ]
    nc = _get_module()
    res = run_bass_kernel_spmd(nc, in_maps, core_ids=list(range(B)))
    LAST_RESULTS = res

    out = np.empty((B, N, F_OUT), dtype=np.float32)
    for b in range(B):
        out[b] = res.results[b]["yt"].T
    return out


# revision 24
# speedup vs baseline: 1.1515x; 1.0404x over previous
"""Multi-head graph-attention layer for Trainium2 (8-core SPMD).

The reference computes per-head projections hp = einsum("bnf,hfd->bhnd", h, W),
dense attention scores e = hp @ hp^T, LeakyReLU, softmax over the last axis,
and then multiplies hp by sum_j(softmax(e))_j. The sum of a softmax over its
own normalization axis is identically 1, so the layer's exact mathematical
output is hp itself (concatenated over heads):

    out[b, n, h*64+d] = sum_f h[b,n,f] * W[h,f,d]  =  (h[b] @ Wc)[n, h*64+d]

with Wc[f, h*64+d] = W[h,f,d]. The reference's deviation from rowsum==1 is
fp32 rounding noise (~1e-6 relative) that no reimplementation reproduces, so
computing the projection directly is both the fastest and the most accurate
realization. `adj` is unused by the reference and is ignored here.

Sharding: data-parallel over the batch dim B=8, one graph per NeuronCore.
Each core computes Y[b]^T = (Wc^T @ h[b]^T) as a [256,256] x [256,2048]
matmul with Wc chunks stationary on the PE (float32r: single-pass reduced-
precision fp32, 1 cycle/row). Inputs are host-transposed to [F_IN, N] so
every DMA is fully contiguous. Warm-up matmuls run on scratch data during
the input-DMA wait so the real matmuls hit the 2.4 GHz warm clock.
"""

import numpy as np

import concourse.bass as bass
import concourse.mybir as mybir
import concourse.tile as tile
from concourse import bacc
from concourse.bass_utils import run_bass_kernel_spmd

B = 8          # graphs == cores
N = 2048       # nodes per graph
F_IN = 256     # input features (= contraction dim K)
F_OUT = 256    # num_heads * d_head
P = 128        # SBUF/PSUM partitions
NTILE = 512    # PSUM bank free-dim (fp32)

KC = F_IN // P     # 2 contraction chunks
MC = F_OUT // P    # 2 output-feature chunks
NC_ = N // NTILE   # 4 node chunks
XSPLIT = 2         # node-dim halves per x DMA
XW = N // XSPLIT   # 1024

N_WARMUP_MM = 8    # scratch matmuls covering the input-DMA wait

# PE matmul dtype: float32 (exact, 4 cycles/row), float32r (reduced-precision
# single pass, 1 cycle/row at N=512, rel err ~1.4e-4), bfloat16 (1 cycle/row,
# half input DMA, rel err ~2.2e-3).
MATMUL_DTYPE = "float32r"

_module_cache = {}

# test.py reads this after calling kernel() to get profile/exec-time info.
LAST_RESULTS = None


def _build_module(mm_dtype: str) -> bass.Bass:
    if mm_dtype == "bfloat16":
        in_dt = mybir.dt.bfloat16
    elif mm_dtype == "float32r":
        in_dt = mybir.dt.float32r
    else:
        in_dt = mybir.dt.float32

    nc = bacc.Bacc(None, target_bir_lowering=False)
    # Host-packed input: xin[f, 0:256] = Wc[f, :], xin[f, 256:] = X^T[f, :].
    xin = nc.dram_tensor("xin", [F_IN, F_OUT + N], in_dt, kind="ExternalInput")
    yt = nc.dram_tensor("yt", [F_OUT, N], mybir.dt.float32, kind="ExternalOutput")
    XOFF = F_OUT  # x columns start here inside a packed row

    with tile.TileContext(nc) as tc:
        with (
            tc.tile_pool(name="wpool", bufs=1) as wpool,
            tc.tile_pool(name="xpool", bufs=1) as xpool,
            tc.tile_pool(name="ypool", bufs=1) as ypool,
            tc.tile_pool(name="warmpool", bufs=1) as warmpool,
            tc.tile_pool(name="drampool", bufs=1, space="DRAM") as dram,
            tc.tile_pool(name="pspool", bufs=1, space="PSUM") as pspool,
        ):
            # Scratch operands for PE warm-up (zeros; values are irrelevant).
            wu = warmpool.tile([P, NTILE], mybir.dt.bfloat16, name="wu", tag="wu")
            nc.gpsimd.memset(wu[:], 0.0)
            wu_mm = wu[:]

            # Per-k packed tiles [128, 2304]: cols 0:256 weights, 256: x.
            # Two single-run DMAs per k so matmuls chase the stream; all on
            # the sync HWDGE queue in need-order.
            CUT = F_OUT + XW  # per-k split: [w | x first half], then rest
            xk_sb = [
                xpool.tile([P, F_OUT + N], in_dt, name=f"xk{k}", tag=f"xk{k}")
                for k in range(KC)
            ]
            # Interleave k chunks so the first node-half of BOTH k's lands
            # early: its accumulation groups then finish while the second
            # half streams, letting output DMAs overlap the input tail.
            nc.sync.dma_start(xk_sb[0][:, :CUT], xin[0:P, :CUT])
            nc.sync.dma_start(xk_sb[1][:, :CUT], xin[P : 2 * P, :CUT])
            nc.sync.dma_start(xk_sb[0][:, CUT:], xin[0:P, CUT:])
            nc.sync.dma_start(xk_sb[1][:, CUT:], xin[P : 2 * P, CUT:])


            ps = [
                [
                    pspool.tile(
                        [P, NTILE], mybir.dt.float32, name=f"ps{m}_{n}", tag=f"ps{m}_{n}"
                    )
                    for n in range(NC_)
                ]
                for m in range(MC)
            ]
            y_sb = [
                ypool.tile([P, N], mybir.dt.float32, name=f"y{m}", tag=f"y{m}")
                for m in range(MC)
            ]

            # PE clock warm-up on scratch data while the x DMAs are in
            # flight. Runs on ps[0][0] before its real accumulation group;
            # Tile's WAW tracking keeps program order.
            for _ in range(N_WARMUP_MM):
                nc.tensor.matmul(ps[0][0][:], wu_mm[:, :P], wu_mm, start=True, stop=True)

            # Node-half outer, then k: each half's groups close right after
            # that half's k=1 chunk lands, so its outputs fly while the next
            # half is still streaming in.
            for half in range(XSPLIT):
                ns = range(half * (NC_ // XSPLIT), (half + 1) * (NC_ // XSPLIT))
                for k in range(KC):
                  for n in ns:
                    for m in range(MC):
                        nc.tensor.matmul(
                            ps[m][n][:],
                            xk_sb[k][:, m * P : (m + 1) * P],
                            xk_sb[k][:, XOFF + n * NTILE : XOFF + (n + 1) * NTILE],
                            start=(k == 0),
                            stop=(k == KC - 1),
                        )
                        if k == KC - 1:
                            # Eviction alternates DVE/ACT; all outputs go on
                            # the single warmed-up HWDGE stream.
                            dst = y_sb[m][:, n * NTILE : (n + 1) * NTILE]
                            yslice = yt[m * P : (m + 1) * P, n * NTILE : (n + 1) * NTILE]
                            if (2 * n + m) % 2 == 0:
                                # DVE evicts; the idle sync engine issues the
                                # store on its (ramped) queue.
                                nc.vector.tensor_copy(dst, ps[m][n][:])
                                nc.sync.dma_start(yslice, dst)
                            else:
                                # ACT evicts and issues its own store on the
                                # scalar queue - no cross-engine stall.
                                nc.scalar.copy(dst, ps[m][n][:])
                                nc.scalar.dma_start(yslice, dst)
    nc.compile()
    return nc


def _get_module() -> bass.Bass:
    if MATMUL_DTYPE not in _module_cache:
        _module_cache[MATMUL_DTYPE] = _build_module(MATMUL_DTYPE)
    return _module_cache[MATMUL_DTYPE]


def kernel(h: np.ndarray, adj: np.ndarray, W: np.ndarray, **_unused) -> np.ndarray:
    global LAST_RESULTS
    h = np.asarray(h, dtype=np.float32)
    W = np.asarray(W, dtype=np.float32)
    # Wc[f, head*64+d] = W[head, f, d]
    wc = np.ascontiguousarray(W.transpose(1, 0, 2).reshape(F_IN, F_OUT))

    if MATMUL_DTYPE == "bfloat16":
        import ml_dtypes

        cast = lambda a: np.ascontiguousarray(a.astype(ml_dtypes.bfloat16))
    else:
        cast = np.ascontiguousarray

    wc_in = cast(wc)
    in_maps = [
        {"xin": np.ascontiguousarray(np.hstack([wc_in, cast(h[b].T)]))}
        for b in range(B)
    ]
    nc = _get_module()
    res = run_bass_kernel_spmd(nc, in_maps, core_ids=list(range(B)))
    LAST_RESULTS = res

    out = np.empty((B, N, F_OUT), dtype=np.float32)
    for b in range(B):
        out[b] = res.results[b]["yt"].T
    return out
